# revision 21
# baseline (speedup 1.0000x reference)
"""DescriptorDiversityLoss on 8 Trainium2 NeuronCores.

Reference computes sim = F F^T (M x M, M = 8192) and returns
|(sum(sim) - trace(sim)) / (M^2 - M)|.

Math identity used (exact in real arithmetic):
    sum(sim)   = || sum_i f_i ||^2           (f_i = rows of F)
    trace(sim) = sum_i ||f_i||^2             (total sum of squares)
so the loss needs one pass over the 8 MiB input: per core (a) column
sums of its row block and (b) its total sum of squares.

Sharding: rows split across 8 cores (1024 rows / 1 MiB each).  The
per-core (1024, 256) block is viewed as (128, 2048) - partition p holds
rows 8p..8p+7 - and streamed in four 512-column chunks so compute
pipelines with the DMA stream.  Column c of the view maps to original
column c % 256, so 256-strided folds preserve column identity:
  - VectorE folds chunks into a running acc (128, 256); a 2-block chunk
    pair-folds into an independent tile first so the latency-bound acc
    chain only grows by one link per chunk.  The host finishes the
    partition/core reduction (~270 KB total, trivial numpy work).
  - Squares for the trace are split across ScalarE (activation Square
    with accum_out -> per-partition row sums) and GpSimd (tensor mult +
    full XYZWC reduce -> scalar) so ScalarE is free when the last chunk
    lands; the host sums the per-segment columns.
Beyond the layout, two framework overheads are patched out: the unused
const-bank memsets + init barrier (~0.6 us) and the second kernel-tail
barrier (~0.26 us); both removals are validated for repeat execution.
"""

import numpy as np

import concourse.bacc as bacc
import concourse.bass as cbass
import concourse.mybir as mybir
import concourse.tile as tile
from concourse.bass_utils import run_bass_kernel_spmd

B, N, D = 16, 512, 256
M = B * N                 # 8192 descriptors total
N_CORES = 8
ROWS = M // N_CORES       # 1024 rows per core
P = 128                   # SBUF partitions
FREE = ROWS * D // P      # 2048 f32 per partition (8 KiB contiguous)

# chunk widths (cols) and per-chunk square split (act_w, pool_w, dve_w)
CFG = {
    "widths": [512, 512, 512, 512],
    "squares": [
        (512, 0, 0),
        (512, 0, 0),
        (256, 256, 0),
        (512, 0, 0),
    ],
}


def _patched_drain_and_barrier(self, tick_clock, wait_clock):
    """Tile kernel tail minus the second all-engine barrier.

    Stock Tile emits drain -> barrier -> sem-clear -> barrier.  The final
    barrier only keeps engines from running past the sem-clears, but each
    engine's stream simply ends here and NRT waits for all engines anyway;
    the clears still complete on their issuing engine.  Dropping it saves
    ~260 ns and repeat executions stay correct (sems are still cleared).
    """
    from concourse.tile import ScopedClock

    drain_inst = self.nc.sync.drain()
    wait_clock.add_sem_waits(
        drain_inst.ins, ScopedClock({None: tick_clock.global_clock})
    )
    self.nc.all_engine_barrier()
    popped = self.nc._tile_sem_poison_stack.pop()
    assert popped is self._sem_poison
    self.nc.clear_and_free_semaphores(list(self.sems.allocated().values()))

_cached_nc = None


def _build_nc(cfg=CFG):
    f32 = mybir.dt.float32
    widths = cfg["widths"]
    squares = cfg["squares"]
    assert sum(widths) == FREE
    n_seg = sum(1 for sp in squares for w in sp if w > 0)
    out_w = D + n_seg

    # Bass.__init__ unconditionally emits a 4-entry const bank via Pool
    # memsets plus an all-engine barrier, and every engine waits on that
    # barrier before starting (~0.6 us).  Only const-float32-0.0 (the Square
    # bias) is read here - and its first reader (ScalarE, gated on the first
    # DMA chunk, ~3 us in) trails the memset by orders of magnitude - so:
    # skip the three unused consts, emit the needed one on the otherwise-idle
    # VectorE, and drop the init barrier entirely.
    orig_memset = cbass.BassGpSimd.memset
    orig_barrier = cbass.Bass.all_engine_barrier

    def patched_memset(self, ap, constant):
        name = getattr(ap.tensor, "name", "")
        if name.startswith(
            ("const-float32-1.0", "const-bfloat16-1.0", "const-uint8-127")
        ):
            return None
        if name.startswith("const-float32-0.0"):
            return self.bass.vector.memset(ap, constant)
        return orig_memset(self, ap, constant)

    cbass.BassGpSimd.memset = patched_memset
    cbass.Bass.all_engine_barrier = lambda self, *a, **k: None
    try:
        nc = bacc.Bacc("TRN2", target_bir_lowering=False, debug=False)
    finally:
        cbass.BassGpSimd.memset = orig_memset
        cbass.Bass.all_engine_barrier = orig_barrier
    x = nc.dram_tensor("x", [P, FREE], f32, kind="ExternalInput")
    out = nc.dram_tensor("out", [P, out_w], f32, kind="ExternalOutput")

    orig_dab = tile.TileContext._drain_and_barrier
    tile.TileContext._drain_and_barrier = _patched_drain_and_barrier
    try:
        _emit_tile_program(nc, widths, squares, out_w, x, out)
    finally:
        tile.TileContext._drain_and_barrier = orig_dab

    nc.compile()
    nc._out_w = out_w
    nc._seg_kinds = _seg_kinds_for(squares)
    return nc


def _seg_kinds_for(squares):
    kinds = []
    for act_w, pool_w, dve_w in squares:
        if act_w:
            kinds.append("full")
        if pool_w:
            kinds.append("scalar")
        if dve_w:
            kinds.append("full")
    return kinds


def _emit_tile_program(nc, widths, squares, out_w, x, out):
    f32 = mybir.dt.float32
    with tile.TileContext(nc) as tc:
        with (
            tc.tile_pool(name="inp", bufs=len(widths)) as ipool,
            tc.tile_pool(name="sq", bufs=3) as qpool,
            tc.tile_pool(name="ufold", bufs=2) as upool,
            tc.tile_pool(name="outp", bufs=1) as opool,
        ):
            o = opool.tile([P, out_w], f32)
            acc = o[:, :D]
            seg = 0          # next rowsq column
            col = 0          # running column offset into x
            first = True
            for j, w in enumerate(widths):
                t = ipool.tile([P, w], f32, tag=f"t{j}")
                nc.sync.dma_start(t[:], x[:, col:col + w])
                col += w

                # fold the chunk's 256-col blocks into acc (VectorE).  The
                # acc chain is latency-bound (~420ns per dependent link), so
                # a 2-block chunk first pair-folds into an independent tile
                # (no chain dependency, issues back-to-back) and merges once.
                n_blk = w // D
                if first:
                    assert n_blk >= 2, "first chunk must have >= 2 blocks"
                    nc.vector.tensor_add(acc, t[:, :D], t[:, D:2 * D])
                    for b in range(2, n_blk):
                        nc.vector.tensor_add(acc, acc, t[:, b * D:(b + 1) * D])
                    first = False
                elif n_blk == 2:
                    u = upool.tile([P, D], f32, tag=f"u{j}")
                    nc.vector.tensor_add(u[:], t[:, :D], t[:, D:2 * D])
                    nc.vector.tensor_add(acc, acc, u[:])
                else:
                    for b in range(n_blk):
                        nc.vector.tensor_add(acc, acc, t[:, b * D:(b + 1) * D])

                # sums of squares, segmented across ACT / Pool / DVE.
                # ACT/DVE deposit per-partition row sums (full column);
                # Pool (no accum_out support) squares then full-reduces to a
                # single scalar in row 0 of its column.
                act_w, pool_w, dve_w = squares[j]
                assert act_w + pool_w + dve_w == w
                off = 0
                for eng_name, ew in (("act", act_w), ("pool", pool_w),
                                     ("dve", dve_w)):
                    if ew == 0:
                        continue
                    src = t[:, off:off + ew]
                    sq = qpool.tile([P, ew], f32, tag=f"sq{seg}")
                    accum = o[:, D + seg:D + seg + 1]
                    if eng_name == "act":
                        nc.scalar.activation(
                            sq[:], src,
                            mybir.ActivationFunctionType.Square,
                            accum_out=accum,
                        )
                    elif eng_name == "pool":
                        nc.gpsimd.tensor_tensor(
                            sq[:], src, src, op=mybir.AluOpType.mult
                        )
                        nc.gpsimd.tensor_reduce(
                            o[:1, D + seg:D + seg + 1], sq[:],
                            axis=mybir.AxisListType.XYZWC,
                            op=mybir.AluOpType.add,
                        )
                    else:
                        nc.vector.scalar_tensor_tensor(
                            sq[:], src, 1.0, src,
                            op0=mybir.AluOpType.mult,
                            op1=mybir.AluOpType.mult,
                            accum_out=accum,
                        )
                    off += ew
                    seg += 1

            nc.sync.dma_start(out[:], o[:])


def kernel(descriptors: np.ndarray) -> np.ndarray:
    global _cached_nc
    if _cached_nc is None:
        _cached_nc = _build_nc()
    nc = _cached_nc

    flat = np.ascontiguousarray(descriptors, dtype=np.float32).reshape(M, D)
    in_maps = [
        {"x": flat[c * ROWS:(c + 1) * ROWS].reshape(P, FREE)}
        for c in range(N_CORES)
    ]
    results = run_bass_kernel_spmd(nc, in_maps, core_ids=list(range(N_CORES)))

    rs = np.stack([r["out"] for r in results.results]).astype(np.float64)
    s = rs[:, :, :D].sum(axis=(0, 1))   # (256,) global column sums
    sumsq = 0.0                         # trace(sim)
    for i, kind in enumerate(nc._seg_kinds):
        col = rs[:, :, D + i]
        sumsq += col.sum() if kind == "full" else col[:, 0].sum()
    off_diag = float(s @ s) - sumsq
    loss = abs(off_diag / (M * (M - 1)))
    return np.float32(loss)
